# revision 28
# baseline (speedup 1.0000x reference)
"""Trainium2 Bass kernel for nn_CQLoss (composite loss function).

Strategy: pure data parallel over batch dim (64 batches -> 8 per core).

All large tensors travel as fp8 (float8e4 / E4M3): the kernel is
DMA-bandwidth-bound (the v2 cost model serializes every transfer on the
shared DMA_ENGINES device at 360 GB/s), so fp8 halves the wire time vs
bf16. fp8 would cripple the DVE (1-byte dtypes lose its 2x/4x modes), so
ALL sum-of-squares compute runs on the otherwise-idle PE via Gram
matmuls accumulated in PSUM:

  sum (a-b)^2 = sum a^2 + sum b^2 - 2*sum a.b
              = diag-sum of [A'A + B'B] - 2 * diag-sum of [A'B]

With fp8 DoubleRow perf mode each matmul contracts TWO adjacent
128-column blocks at 0.5 cycles/row: per 256-column block-pair one
"self" matmul accumulates A'A (one more for B'B) and one "cross" matmul
A'B. Self and cross Grams of a group live in ONE [128,256] psum bank;
the diagonals are pulled with a single DVE multiply against a packed
[I | -2*I] mask followed by a tensor_scalar row-sum, which yields
diag(self) - 2*diag(cross) — the group's fused partial — in one column.

The recon term (rzs gathered rows vs zs) and the pts term (gathered pts
vs pts_gt, landmark-weighted) are fused into the SAME psum banks: pts
values are host-prescaled by sqrt(w_p * D/PC) so both terms share the
1/(B*S*D) normalizer; a global lambda=0.5 prescale keeps the weighted
landmark values inside fp8e4's +-240 range. Rows gathered by `mapping`
ride one indirect DMA per batch straight from HBM ([rzs_row | scaled
pts_row | pad]); the dense side ([zs | scaled pts_gt | pad]) is
host-packed into identically-shaped rows so the block-pair APs line up.

Batches 0..5 accumulate in psum group 1 (extracted while batches 6/7
are still streaming); batches 6+7 go to group 2, with batch 7 split
into three column chunks (5+2+2 block-pairs) so the serial tail after
the last DMA byte is short. The single output DMA is issued early with
the group-2 gate ATTACHED to the instruction (walrus requires DGE sync
info anyway), so its SEQ phase overlaps the wait; the gate is the
group-2 extract's multiply (not its accumulate): the accumulate retires
~0.35us after the gate while the DMA's HWDGE+DGE descriptor chain takes
>1.2us before the transfer reads the accumulator, so the ordering holds
with ~1us of margin on both the cost model and hardware. Nothing waits
on the output DMA's completion semaphore - the runtime quiesces all DMA
rings at NEFF completion.

KL term: qy is sent as q^ = V*qy (fp8), ACT computes L = ln(q^ + V*eps)
(fp8 out), and PE cross-Grams q^ against L: sum q^*L = V*sum q*(ln(q+e)
- ln(1/V)). The best term is tiny and stays in exact f32: its constants
ride a late small DMA whose short Pool-sub + ACT-square chain overlaps
the last chunk's PE chain instead of idling early.

Raw bass (explicit semaphores): at most one attached sync-wait per
compute instruction, so waits are standalone wait_ge ops; one semaphore
per DMA; per-batch zs DMAs are staggered off earlier completions so the
DMA_ENGINES stream interleaves them with the (Pool-paced) gathers and
the PE is fed a batch at a time. Host does the final cheap reduction in
float64.
"""

import os
import sys

import numpy as np

for _p in ("/opt/trn_rl_repo", "/root/.axon_site/_ro/trn_rl_repo"):
    if os.path.isdir(_p) and _p not in sys.path:
        sys.path.insert(0, _p)

B, S, D, P, C, V = 64, 128, 2048, 118, 2, 512
PC = P * C  # 236
PAD = 20
ROW = D + PC + PAD  # 2304 = 9 * 256 (DR fp8 needs 128-wide weight tiles)
N_CORES = 8
BL = B // N_CORES  # 8 batches per core
ALPHA, BETA, GAMMA, EPS = 10.0, 0.1, 1.0, 1e-20
MARKS = (0, 29, 88, 117)
W_MARK = ALPHA * PC / (len(MARKS) * C)  # 295.0
LAM = 0.5  # global prescale: keeps sqrt(w*D/PC)-scaled pts under fp8e4 max 240

NPAIR = ROW // 256  # 9 block-pairs per batch
CH7 = (5, 2, 2)  # batch-7 chunk sizes in block-pairs (512B chunks: no 2x DMA penalty)
KPAIR = BL * V // 256  # 16 KL block-pairs

# cpack (f32 cols): 0..6 abs mapping batches 0..6 (int32 bits), 7 batch-7
# mapping (rebased, int32 bits), 8 ln bias, 9..24 w*best, 25..40 w*best_gt,
# 41..168 [I | -2I] mask (bf16 bits); 169 f32 cols = 676B contiguous run
NCONST = 169
BC = BL * C  # 16

_CACHE: dict = {}


def _build_bass(vector_dims: int):
    import concourse.bass as bass
    from concourse import mybir

    f32 = mybir.dt.float32
    bf16 = mybir.dt.bfloat16
    f8 = mybir.dt.float8e4
    i32 = mybir.dt.int32
    Act = mybir.ActivationFunctionType
    Alu = mybir.AluOpType
    DR = mybir.MatmulPerfMode.DoubleRow

    nc = bass.Bass()

    gath = nc.dram_tensor("gath", [7 * S, ROW], f8, kind="ExternalInput")
    _c = []
    _a = 0
    for n in CH7:
        _b = min(_a + n * 256, ROW)
        _c.append(_b - _a)
        _a = _b
    g7 = [
        nc.dram_tensor(f"g7{i}", [S, w], f8, kind="ExternalInput")
        for i, w in enumerate(_c)
    ]
    zsg = nc.dram_tensor("zsg", [S, BL * ROW], f8, kind="ExternalInput")
    qy8 = nc.dram_tensor("qy8", [S, BL * V], f8, kind="ExternalInput")
    cpack = nc.dram_tensor("cpack", [S, NCONST], f32, kind="ExternalInput")
    cbest = nc.dram_tensor("cbest", [S, 2 * BC], f32, kind="ExternalInput")
    # po cols: 0=fused group1, 1=kl, 2=best, 3=fused group2
    po = nc.dram_tensor("po", [S, 4], f32, kind="ExternalOutput")

    from contextlib import ExitStack

    with ExitStack() as ctx:
        ga_t = ctx.enter_context(nc.sbuf_tensor([S, BL * ROW], f8))
        zb_t = ctx.enter_context(nc.sbuf_tensor([S, BL * ROW], f8))
        qy_t = ctx.enter_context(nc.sbuf_tensor([S, BL * V], f8))
        lq_t = ctx.enter_context(nc.sbuf_tensor([S, BL * V], f8))
        cp_t = ctx.enter_context(nc.sbuf_tensor([S, NCONST], f32))
        bd_t = ctx.enter_context(nc.sbuf_tensor([S, BC], f32))
        scr_t = ctx.enter_context(nc.sbuf_tensor([S, 3 * 256], f32))
        acc_t = ctx.enter_context(nc.sbuf_tensor([S, 4], f32))

        ps_g1 = ctx.enter_context(nc.psum_tensor([128, 256], f32))
        ps_g2 = ctx.enter_context(nc.psum_tensor([128, 256], f32))
        ps_kl = ctx.enter_context(nc.psum_tensor([128, 128], f32))

        sem_cp = ctx.enter_context(nc.semaphore("sem_cp"))
        sem_qy = ctx.enter_context(nc.semaphore("sem_qy"))
        sem_z = [ctx.enter_context(nc.semaphore(f"sem_z{b}")) for b in range(10)]
        sem_g = [ctx.enter_context(nc.semaphore(f"sem_g{b}")) for b in range(10)]
        sem_lnq = ctx.enter_context(nc.semaphore("sem_lnq"))
        sem_pe1 = ctx.enter_context(nc.semaphore("sem_pe1"))
        sem_peK = ctx.enter_context(nc.semaphore("sem_peK"))
        sem_pe2 = ctx.enter_context(nc.semaphore("sem_pe2"))
        sem_bsub = ctx.enter_context(nc.semaphore("sem_bsub"))
        sem_cb = ctx.enter_context(nc.semaphore("sem_cb"))
        cb_t = ctx.enter_context(nc.sbuf_tensor([S, 2 * BC], f32))
        sem_bsq = ctx.enter_context(nc.semaphore("sem_bsq"))
        sem_mul = ctx.enter_context(nc.semaphore("sem_mul"))
        sem_x1 = ctx.enter_context(nc.semaphore("sem_x1"))
        sem_x2 = ctx.enter_context(nc.semaphore("sem_x2"))
        block = ctx.enter_context(nc.Block())

        map_i = cp_t[:, 0:BL].bitcast(i32)
        mask = cp_t[:, 41:169].bitcast(bf16)  # [128, 256] = [I | -2I]

        def pair(t, col, n=256):
            # [128, 2, n/2] fp8 view of two adjacent column blocks
            return t[:, col : col + n].rearrange("p (two m) -> p two m", two=2)

        # batch-7 z/g chunk column ranges (within the batch-7 region)
        ch7_cols = []
        c0 = 0
        for n in CH7:
            c1 = min(c0 + n * 256, ROW)
            ch7_cols.append((c0, c1))
            c0 = c1

        @block.sync
        def _(sync):
            sync.dma_start(
                out=zb_t[:, 0:ROW], in_=zsg[:, 0:ROW]
            ).then_inc(sem_z[0], 16)
            sync.dma_start(out=cp_t[:], in_=cpack[:]).then_inc(sem_cp, 16)
            sync.dma_start(out=qy_t[:], in_=qy8[:]).then_inc(sem_qy, 16)
            sync.dma_start(
                out=zb_t[:, ROW : 2 * ROW], in_=zsg[:, ROW : 2 * ROW]
            ).then_inc(sem_z[1], 16)
            # stagger the remaining chunks so the shared DMA engines weave
            # them between the (Pool-paced) gathers batch by batch
            plan = [
                (sem_z[0], 2 * ROW, 3 * ROW, sem_z[2]),
                (sem_qy, 3 * ROW, 4 * ROW, sem_z[3]),
                (sem_z[1], 4 * ROW, 5 * ROW, sem_z[4]),
                (sem_z[2], 5 * ROW, 6 * ROW, sem_z[5]),
                (sem_z[3], 6 * ROW, 7 * ROW, sem_z[6]),
                (sem_z[4], 7 * ROW + ch7_cols[0][0], 7 * ROW + ch7_cols[0][1],
                 sem_z[7]),
                (sem_z[5], 7 * ROW + ch7_cols[1][0], 7 * ROW + ch7_cols[1][1],
                 sem_z[8]),
                (sem_z[5], 7 * ROW + ch7_cols[2][0], 7 * ROW + ch7_cols[2][1],
                 sem_z[9]),  # chunk DMAs share sems with their gathers (wait >=32)
            ]
            for k, (gate, c0_, c1_, sem) in enumerate(plan):
                sync.wait_ge(gate, 16)
                sync.dma_start(
                    out=zb_t[:, c0_:c1_], in_=zsg[:, c0_:c1_]
                ).then_inc(sem, 16)
            # best-term constants land LAST: their short Pool+ACT chain
            # (sub, square) overlaps the last chunk's PE+extract chain, so
            # both gates into the output DMA fire at about the same time
            sync.dma_start(out=cb_t[:], in_=cbest[:]).then_inc(sem_cb, 16)
            # single output DMA once every partial has landed in acc; the
            # runtime syncs all DMA rings at NEFF completion, so no explicit
            # completion wait is needed before program end. Gated on the
            # group-2 extract's multiply (see module docstring re margin).
            sync.wait_ge(sem_x1, 2)
            sync.wait_ge(sem_bsq, 1)
            sync.dma_start(out=po[:], in_=acc_t[:])._wait_ge(sem_mul, 3).then_inc(
                sem_x2, 16
            )

        @block.gpsimd
        def _(gpsimd):
            gpsimd.wait_ge(sem_cp, 16)  # mapping loaded
            for b in range(7):
                gpsimd.indirect_dma_start(
                    out=ga_t[:, b * ROW : (b + 1) * ROW],
                    out_offset=None,
                    in_=gath[:],
                    in_offset=bass.IndirectOffsetOnAxis(
                        ap=map_i[:, b : b + 1], axis=0
                    ),
                ).then_inc(sem_g[b], 16)
            for i in range(len(CH7)):
                c0_, c1_ = ch7_cols[i]
                gpsimd.indirect_dma_start(
                    out=ga_t[:, 7 * ROW + c0_ : 7 * ROW + c1_],
                    out_offset=None,
                    in_=g7[i][:],
                    in_offset=bass.IndirectOffsetOnAxis(ap=map_i[:, 7:8], axis=0),
                ).then_inc(sem_z[7 + i], 16)
            gpsimd.wait_ge(sem_cb, 16)
            nc.gpsimd.tensor_sub(
                bd_t[:], cb_t[:, 0:BC], cb_t[:, BC : 2 * BC]
            ).then_inc(sem_bsub, 1)

        @block.tensor
        def _(tensor):
            def pairs(ps, cols, s_start, c_start, s_stop, c_stop, inc=None):
                n = len(cols)
                for i, (col, w) in enumerate(cols):
                    m = w // 2
                    a_p, b_p = pair(ga_t, col, w), pair(zb_t, col, w)
                    first, last = i == 0, i == n - 1
                    nc.tensor.matmul(
                        ps[0:m, 0:m], a_p, a_p,
                        start=s_start and first, stop=False, perf_mode=DR,
                    )
                    nc.tensor.matmul(
                        ps[0:m, 0:m], b_p, b_p,
                        start=False, stop=s_stop and last, perf_mode=DR,
                    )
                    nc.tensor.matmul(
                        ps[0:m, 128 : 128 + m], a_p, b_p,
                        start=c_start and first, stop=c_stop and last,
                        perf_mode=DR,
                    )
                if inc is not None:
                    # zero-cost engine op ordered after the last matmul: its
                    # sem fires without the matmul's trailing write-ack delay
                    nc.tensor.ldweights(
                        weights=pair(ga_t, cols[-1][0], cols[-1][1]),
                        perf_mode=DR,
                    ).then_inc(inc, 1)

            def bcols(b, j0, j1):
                return [(b * ROW + j * 256, 256) for j in range(j0, j1)]

            for b in range(5):
                tensor.wait_ge(sem_g[b], 16)
                tensor.wait_ge(sem_z[b], 16)
                pairs(ps_g1, bcols(b, 0, NPAIR), b == 0, b == 0, False, False)
            # KL cross-Gram: lnq is ready long before batch 5's data
            tensor.wait_ge(sem_lnq, 1)
            for k in range(KPAIR):
                mm = nc.tensor.matmul(
                    ps_kl[:], pair(qy_t, k * 256), pair(lq_t, k * 256),
                    start=k == 0, stop=k == KPAIR - 1, perf_mode=DR,
                )
            mm.then_inc(sem_peK, 1)
            tensor.wait_ge(sem_g[5], 16)
            tensor.wait_ge(sem_z[5], 16)
            pairs(ps_g1, bcols(5, 0, NPAIR), False, False, True, True,
                  inc=sem_pe1)
            tensor.wait_ge(sem_g[6], 16)
            tensor.wait_ge(sem_z[6], 16)
            pairs(ps_g2, bcols(6, 0, NPAIR), True, True, False, False)
            j0 = 0
            last = len(CH7) - 1
            for i, n in enumerate(CH7):
                tensor.wait_ge(sem_z[7 + i], 32)  # z chunk + gather share a sem
                pairs(ps_g2, bcols(7, j0, j0 + n), False, False,
                      i == last, i == last, inc=sem_pe2 if i == last else None)
                j0 += n

        @block.scalar
        def _(scalar):
            scalar.wait_ge(sem_qy, 16)
            scalar.wait_ge(sem_cp, 16)
            nc.scalar.activation(
                lq_t[:], qy_t[:], Act.Ln, bias=cp_t[:, 8:9], scale=1.0
            ).then_inc(sem_lnq, 1)
            # best term: acc[:,2] = per-partition sum(bd^2), exact f32
            scalar.wait_ge(sem_bsub, 1)
            nc.scalar.activation(
                bd_t[:], bd_t[:], Act.Square, accum_out=acc_t[:, 2:3]
            ).then_inc(sem_bsq, 1)


        @block.vector
        def _(vector):
            state = {"nmul": 0}

            def extract(ps, width, slot, accum, sem):
                # diag(self) - 2*diag(cross) via the packed [I | -2I] mask
                scr = scr_t[:, slot * 256 : slot * 256 + width]
                nc.vector.tensor_mul(
                    scr, ps[:], mask[:, 0:width]
                ).then_inc(sem_mul, 1)
                state["nmul"] += 1
                vector.wait_ge(sem_mul, state["nmul"])  # same-engine RAW
                nc.vector.tensor_scalar(
                    out=scr,
                    in0=scr,
                    scalar1=1.0,
                    scalar2=0.0,
                    op0=Alu.mult,
                    op1=Alu.add,
                    accum_out=accum,
                ).then_inc(sem, 1)

            vector.wait_ge(sem_pe1, 1)
            extract(ps_g1, 256, 0, acc_t[:, 0:1], sem_x1)
            vector.wait_ge(sem_peK, 1)
            extract(ps_kl, 128, 1, acc_t[:, 1:2], sem_x1)
            vector.wait_ge(sem_pe2, 1)
            extract(ps_g2, 256, 2, acc_t[:, 3:4], sem_x2)

    return nc


def _get_nc(vector_dims: int):
    key = ("nc", vector_dims)
    if key not in _CACHE:
        _CACHE[key] = _build_bass(vector_dims)
    return _CACHE[key]


def _prepare(inputs):
    import ml_dtypes

    f8 = ml_dtypes.float8_e4m3
    bf16 = ml_dtypes.bfloat16

    zs = np.asarray(inputs["zs"], dtype=np.float32)
    rzs = np.asarray(inputs["rzs"], dtype=np.float32)
    pts = np.asarray(inputs["pts"], dtype=np.float32)
    pts_gt = np.asarray(inputs["pts_gt"], dtype=np.float32)
    qy = np.asarray(inputs["qy"], dtype=np.float32)
    best = np.asarray(inputs["best"], dtype=np.float64)
    best_gt = np.asarray(inputs["best_gt"], dtype=np.float64)
    mapping = np.asarray(inputs["mapping"])
    vector_dims = int(np.asarray(inputs["vector_dims"]))

    w_p = np.ones(P, dtype=np.float64)
    w_p[list(MARKS)] += W_MARK
    w_sq = np.sqrt(w_p)  # (118,) for the best term (exact f32 path)
    s_pt = (LAM * np.sqrt(w_p * D / PC)).astype(np.float32)  # fused-bank scale

    # region rows: [lam*rz | s_pt*pts | 0pad]  /  [lam*zs | s_pt*pts_gt | 0pad]
    gath8 = np.zeros((B, S, ROW), dtype=f8)
    gath8[:, :, :D] = (rzs * LAM).astype(f8)
    gath8[:, :, D : D + PC] = (pts * s_pt[None, None, :, None]).reshape(
        B, S, PC
    ).astype(f8)
    zsg8 = np.zeros((B, S, ROW), dtype=f8)
    zsg8[:, :, :D] = (zs * LAM).astype(f8)
    zsg8[:, :, D : D + PC] = (pts_gt * s_pt[None, None, :, None]).reshape(
        B, S, PC
    ).astype(f8)
    qy8 = np.ascontiguousarray((qy * np.float32(vector_dims)).astype(f8))

    best_w = (best * w_sq[None, :, None]).astype(np.float32)
    bestgt_w = (best_gt * w_sq[None, :, None]).astype(np.float32)
    mask = np.zeros((128, 256), dtype=bf16)
    mask[:, 0:128] = np.eye(128, dtype=bf16)
    mask[:, 128:256] = (-2.0 * np.eye(128)).astype(bf16)
    mask_bits = mask.view(np.float32)  # (128, 128)

    base = (np.arange(7, dtype=np.int32) * S)[:, None]  # abs offsets, b 0..6

    ch7_cols = []
    c0 = 0
    for n in CH7:
        c1 = min(c0 + n * 256, ROW)
        ch7_cols.append((c0, c1))
        c0 = c1

    in_maps = []
    for c in range(N_CORES):
        sl = slice(c * BL, (c + 1) * BL)
        m = mapping[sl].astype(np.int32)  # (8, S)
        cpk = np.zeros((S, NCONST), dtype=np.float32)
        cpk[:, 0:7] = np.ascontiguousarray((m[:7] + base).T).view(np.float32)
        cpk[:, 7] = np.ascontiguousarray(m[7]).view(np.float32)
        cpk[:, 8] = np.float32(vector_dims * EPS)
        cpk[:, 41:169] = mask_bits
        cbk = np.zeros((S, 2 * BC), dtype=np.float32)
        cbk[:P, 0:BC] = best_w[sl].transpose(1, 0, 2).reshape(P, BC)
        cbk[:P, BC : 2 * BC] = bestgt_w[sl].transpose(1, 0, 2).reshape(P, BC)
        g8 = gath8[sl]  # (8, S, ROW)
        im = {
            "gath": g8[:7].reshape(7 * S, ROW),
            "zsg": np.ascontiguousarray(
                zsg8[sl].transpose(1, 0, 2).reshape(S, BL * ROW)
            ),
            "qy8": np.ascontiguousarray(
                qy8[sl].transpose(1, 0, 2).reshape(S, BL * V)
            ),
            "cpack": cpk,
            "cbest": cbk,
        }
        for i, (c0_, c1_) in enumerate(ch7_cols):
            im[f"g7{i}"] = np.ascontiguousarray(g8[7, :, c0_:c1_])
        in_maps.append(im)
    return in_maps, vector_dims


def _combine(results, vector_dims) -> np.ndarray:
    s_fused = np.float64(0.0)
    s_kl = np.float64(0.0)
    s_best = np.float64(0.0)
    for r in results:
        por = r["po"].astype(np.float64)
        s_fused += por[:, 0].sum() + por[:, 3].sum()
        s_kl += por[:, 1].sum()
        s_best += por[:, 2].sum()

    recon_pts = s_fused / (LAM * LAM * B * S * D)
    kld = s_kl / (vector_dims * B * S)
    best_term = s_best / (B * PC)
    total = BETA * kld + GAMMA * recon_pts + best_term
    return np.float32(total)


def kernel(**inputs) -> np.ndarray:
    from concourse.bass_utils import run_bass_kernel_spmd

    in_maps, vector_dims = _prepare(inputs)
    nc = _get_nc(vector_dims)

    trace = os.environ.get("KERNEL_TRACE", "") == "1"
    res = run_bass_kernel_spmd(nc, in_maps, core_ids=list(range(N_CORES)), trace=trace)
    if trace and res.exec_time_ns is not None:
        print(f"HW exec time: {res.exec_time_ns} ns")
        if res.instructions_and_trace is not None:
            print(f"trace: {res.instructions_and_trace[1]}")

    return _combine(res.results, vector_dims)


# revision 39
# speedup vs baseline: 1.0454x; 1.0454x over previous
"""Trainium2 Bass kernel for nn_CQLoss (composite loss function).

Strategy: pure data parallel over batch dim (64 batches -> 8 per core).

All large tensors travel as fp8 (float8e4 / E4M3): the kernel is
DMA-bandwidth-bound (the v2 cost model serializes every transfer on the
shared DMA_ENGINES device at 360 GB/s), so fp8 halves the wire time vs
bf16. fp8 would cripple the DVE (1-byte dtypes lose its 2x/4x modes), so
ALL sum-of-squares compute runs on the otherwise-idle PE via Gram
matmuls accumulated in PSUM:

  sum (a-b)^2 = sum a^2 + sum b^2 - 2*sum a.b
              = diag-sum of [A'A + B'B] - 2 * diag-sum of [A'B]

With fp8 DoubleRow perf mode each matmul contracts TWO adjacent
128-column blocks at 0.5 cycles/row: per 256-column block-pair one
"self" matmul accumulates A'A (one more for B'B) and one "cross" matmul
A'B. Self and cross Grams of a group live in ONE [128,256] psum bank;
the diagonals are pulled with a single DVE multiply against a packed
[I | -2*I] mask followed by a tensor_scalar row-sum, which yields
diag(self) - 2*diag(cross) — the group's fused partial — in one column.

The recon term (rzs gathered rows vs zs) and the pts term (gathered pts
vs pts_gt, landmark-weighted) are fused into the SAME psum banks: pts
values are host-prescaled by sqrt(w_p * D/PC) so both terms share the
1/(B*S*D) normalizer; a global lambda=0.5 prescale keeps the weighted
landmark values inside fp8e4's +-240 range. Rows gathered by `mapping`
ride one indirect DMA per batch straight from HBM ([rzs_row | scaled
pts_row | pad]); the dense side ([zs | scaled pts_gt | pad]) is
host-packed into identically-shaped rows so the block-pair APs line up.

Batches 0..5 accumulate in psum group 1 (extracted while batches 6/7
are still streaming); batches 6+7 go to group 2, with batch 7 split
into three column chunks (5+2+2 block-pairs) so the serial tail after
the last DMA byte is short. The single output DMA is issued early with
the group-2 gate ATTACHED to the instruction (walrus requires DGE sync
info anyway), so its SEQ phase overlaps the wait; the gate is the
group-2 extract's multiply (not its accumulate): the accumulate retires
~0.35us after the gate while the DMA's HWDGE+DGE descriptor chain takes
>1.2us before the transfer reads the accumulator, so the ordering holds
with ~1us of margin on both the cost model and hardware. Nothing waits
on the output DMA's completion semaphore - the runtime quiesces all DMA
rings at NEFF completion.

KL term: qy is sent as q^ = V*qy (fp8), ACT computes L = ln(q^ + V*eps)
(fp8 out), and PE cross-Grams q^ against L: sum q^*L = V*sum q*(ln(q+e)
- ln(1/V)). The best term is tiny and stays in exact f32: its constants
ride a late small DMA whose short Pool-sub + ACT-square chain overlaps
the last chunk's PE chain instead of idling early.

Raw bass (explicit semaphores): at most one attached sync-wait per
compute instruction, so waits are standalone wait_ge ops; one semaphore
per DMA; per-batch zs DMAs are staggered off earlier completions so the
DMA_ENGINES stream interleaves them with the (Pool-paced) gathers and
the PE is fed a batch at a time. Host does the final cheap reduction in
float64.
"""

import os
import sys

import numpy as np

for _p in ("/opt/trn_rl_repo", "/root/.axon_site/_ro/trn_rl_repo"):
    if os.path.isdir(_p) and _p not in sys.path:
        sys.path.insert(0, _p)

B, S, D, P, C, V = 64, 128, 2048, 118, 2, 512
PC = P * C  # 236
PAD = 20
ROW = D + PC + PAD  # 2304 = 9 * 256 (DR fp8 needs 128-wide weight tiles)
N_CORES = 8
BL = B // N_CORES  # 8 batches per core
ALPHA, BETA, GAMMA, EPS = 10.0, 0.1, 1.0, 1e-20
MARKS = (0, 29, 88, 117)
W_MARK = ALPHA * PC / (len(MARKS) * C)  # 295.0
LAM = 0.5  # global prescale: keeps sqrt(w*D/PC)-scaled pts under fp8e4 max 240

NPAIR = ROW // 256  # 9 block-pairs per batch
CH7 = (5, 2, 2)  # batch-7 chunk sizes in block-pairs (512B chunks: no 2x DMA penalty)
KPAIR = BL * V // 256  # 16 KL block-pairs

# cpack (f32 cols): 0..6 abs mapping batches 0..6 (int32 bits), 7 batch-7
# mapping (rebased, int32 bits), 8 ln bias, 9..24 w*best, 25..40 w*best_gt,
# 41..168 [I | -2I] mask (bf16 bits); 169 f32 cols = 676B contiguous run
NCONST = 169
BC = BL * C  # 16

_CACHE: dict = {}


def _build_bass(vector_dims: int):
    import concourse.bass as bass
    from concourse import mybir

    f32 = mybir.dt.float32
    bf16 = mybir.dt.bfloat16
    f8 = mybir.dt.float8e4
    i32 = mybir.dt.int32
    Act = mybir.ActivationFunctionType
    Alu = mybir.AluOpType
    DR = mybir.MatmulPerfMode.DoubleRow

    # bass.Bass() pre-registers four const APs via gpsimd.memset in its
    # init preamble; walrus' verifier reports them as having no reader in
    # this kernel (Square gets an explicit zero bias below), and the four
    # serialized Memsets delay the engine-init barrier ~0.4us. Suppress
    # them during construction only; this kernel issues no memsets itself.
    _gp_cls = type(bass.Bass().gpsimd)
    _orig_memset = _gp_cls.memset
    _gp_cls.memset = lambda self, ap, constant: None
    # The init-time all_engine_barrier only orders per-engine register
    # preambles against OTHER engines' bodies; every engine's body follows
    # its own preamble in program order, semaphores are runtime-reset, and
    # DGE config registers are per-engine, so it is dead weight (~430ns of
    # every engine idling). Suppress during construction; the Block exit
    # barrier (emitted later, outside this window) is untouched.
    _orig_barrier = bass.Bass.all_engine_barrier
    bass.Bass.all_engine_barrier = lambda self, *a, **k: None
    try:
        nc = bass.Bass()
    finally:
        _gp_cls.memset = _orig_memset
        bass.Bass.all_engine_barrier = _orig_barrier

    gath = nc.dram_tensor("gath", [7 * S, ROW], f8, kind="ExternalInput")
    _c = []
    _a = 0
    for n in CH7:
        _b = min(_a + n * 256, ROW)
        _c.append(_b - _a)
        _a = _b
    g7 = [
        nc.dram_tensor(f"g7{i}", [S, w], f8, kind="ExternalInput")
        for i, w in enumerate(_c)
    ]
    zsg = nc.dram_tensor("zsg", [S, BL * ROW], f8, kind="ExternalInput")
    qy8 = nc.dram_tensor("qy8", [S, BL * V], f8, kind="ExternalInput")
    cpack = nc.dram_tensor("cpack", [S, NCONST], f32, kind="ExternalInput")
    cbest = nc.dram_tensor("cbest", [S, 2 * BC], f32, kind="ExternalInput")
    # po cols: 0=fused group1, 1=kl, 2=best, 3=fused group2
    po = nc.dram_tensor("po", [S, 4], f32, kind="ExternalOutput")

    from contextlib import ExitStack

    with ExitStack() as ctx:
        ga_t = ctx.enter_context(nc.sbuf_tensor([S, BL * ROW], f8))
        zb_t = ctx.enter_context(nc.sbuf_tensor([S, BL * ROW], f8))
        qy_t = ctx.enter_context(nc.sbuf_tensor([S, BL * V], f8))
        lq_t = ctx.enter_context(nc.sbuf_tensor([S, BL * V], f8))
        cp_t = ctx.enter_context(nc.sbuf_tensor([S, NCONST], f32))
        bd_t = ctx.enter_context(nc.sbuf_tensor([S, BC], f32))
        scr_t = ctx.enter_context(nc.sbuf_tensor([S, 3 * 256], f32))
        acc_t = ctx.enter_context(nc.sbuf_tensor([S, 4], f32))

        ps_g1 = ctx.enter_context(nc.psum_tensor([128, 256], f32))
        ps_g2 = ctx.enter_context(nc.psum_tensor([128, 256], f32))
        ps_kl = ctx.enter_context(nc.psum_tensor([128, 128], f32))

        sem_cp = ctx.enter_context(nc.semaphore("sem_cp"))
        sem_qy = ctx.enter_context(nc.semaphore("sem_qy"))
        sem_z = [ctx.enter_context(nc.semaphore(f"sem_z{b}")) for b in range(10)]
        sem_g = [ctx.enter_context(nc.semaphore(f"sem_g{b}")) for b in range(10)]
        sem_lnq = ctx.enter_context(nc.semaphore("sem_lnq"))
        sem_pe1 = ctx.enter_context(nc.semaphore("sem_pe1"))
        sem_peK = ctx.enter_context(nc.semaphore("sem_peK"))
        sem_pe2 = ctx.enter_context(nc.semaphore("sem_pe2"))
        sem_bsub = ctx.enter_context(nc.semaphore("sem_bsub"))
        sem_cb = ctx.enter_context(nc.semaphore("sem_cb"))
        cb_t = ctx.enter_context(nc.sbuf_tensor([S, 2 * BC], f32))
        sem_bsq = ctx.enter_context(nc.semaphore("sem_bsq"))
        sem_mul = ctx.enter_context(nc.semaphore("sem_mul"))
        sem_x1 = ctx.enter_context(nc.semaphore("sem_x1"))
        sem_x2 = ctx.enter_context(nc.semaphore("sem_x2"))
        block = ctx.enter_context(nc.Block())

        map_i = cp_t[:, 0:BL].bitcast(i32)
        mask = cp_t[:, 41:169].bitcast(bf16)  # [128, 256] = [I | -2I]

        def pair(t, col, n=256):
            # [128, 2, n/2] fp8 view of two adjacent column blocks
            return t[:, col : col + n].rearrange("p (two m) -> p two m", two=2)

        # batch-7 z/g chunk column ranges (within the batch-7 region)
        ch7_cols = []
        c0 = 0
        for n in CH7:
            c1 = min(c0 + n * 256, ROW)
            ch7_cols.append((c0, c1))
            c0 = c1

        @block.sync
        def _(sync):
            sync.dma_start(
                out=zb_t[:, 0:ROW], in_=zsg[:, 0:ROW]
            ).then_inc(sem_z[0], 16)
            sync.dma_start(out=cp_t[:], in_=cpack[:]).then_inc(sem_cp, 16)
            sync.dma_start(out=qy_t[:], in_=qy8[:]).then_inc(sem_qy, 16)
            sync.dma_start(
                out=zb_t[:, ROW : 2 * ROW], in_=zsg[:, ROW : 2 * ROW]
            ).then_inc(sem_z[1], 16)
            # stagger the remaining chunks so the shared DMA engines weave
            # them between the (Pool-paced) gathers batch by batch
            plan = [
                (sem_z[0], 2 * ROW, 3 * ROW, sem_z[2]),
                (sem_z[1], 3 * ROW, 4 * ROW, sem_z[3]),
                (sem_z[1], 4 * ROW, 5 * ROW, sem_z[4]),
                (sem_z[2], 5 * ROW, 6 * ROW, sem_z[5]),
                (sem_z[3], 6 * ROW, 7 * ROW, sem_z[6]),
                (sem_z[4], 7 * ROW + ch7_cols[0][0], 7 * ROW + ch7_cols[0][1],
                 sem_z[7]),
                (sem_z[5], 7 * ROW + ch7_cols[1][0], 7 * ROW + ch7_cols[1][1],
                 sem_z[8]),
                (sem_z[5], 7 * ROW + ch7_cols[2][0], 7 * ROW + ch7_cols[2][1],
                 sem_z[9]),  # chunk DMAs share sems with their gathers (wait >=32)
            ]
            for k, (gate, c0_, c1_, sem) in enumerate(plan):
                sync.wait_ge(gate, 16)
                sync.dma_start(
                    out=zb_t[:, c0_:c1_], in_=zsg[:, c0_:c1_]
                ).then_inc(sem, 16)
            # best-term constants land LAST: their short Pool+ACT chain
            # (sub, square) overlaps the last chunk's PE+extract chain, so
            # both gates into the output DMA fire at about the same time
            sync.dma_start(out=cb_t[:], in_=cbest[:]).then_inc(sem_cb, 16)
            # single output DMA once every partial has landed in acc; the
            # runtime syncs all DMA rings at NEFF completion, so no explicit
            # completion wait is needed before program end. Gated on the
            # group-2 extract's multiply (see module docstring re margin).
            sync.wait_ge(sem_x1, 2)
            sync.wait_ge(sem_bsq, 1)
            sync.dma_start(out=po[:], in_=acc_t[:])._wait_ge(sem_mul, 3).then_inc(
                sem_x2, 16
            )

        @block.gpsimd
        def _(gpsimd):
            gpsimd.wait_ge(sem_cp, 16)  # mapping loaded
            for b in range(7):
                gpsimd.indirect_dma_start(
                    out=ga_t[:, b * ROW : (b + 1) * ROW],
                    out_offset=None,
                    in_=gath[:],
                    in_offset=bass.IndirectOffsetOnAxis(
                        ap=map_i[:, b : b + 1], axis=0
                    ),
                ).then_inc(sem_g[b], 16)
            for i in range(len(CH7)):
                c0_, c1_ = ch7_cols[i]
                gpsimd.indirect_dma_start(
                    out=ga_t[:, 7 * ROW + c0_ : 7 * ROW + c1_],
                    out_offset=None,
                    in_=g7[i][:],
                    in_offset=bass.IndirectOffsetOnAxis(ap=map_i[:, 7:8], axis=0),
                ).then_inc(sem_z[7 + i], 16)
            gpsimd.wait_ge(sem_cb, 16)
            nc.gpsimd.tensor_sub(
                bd_t[:], cb_t[:, 0:BC], cb_t[:, BC : 2 * BC]
            ).then_inc(sem_bsub, 1)

        @block.tensor
        def _(tensor):
            def pairs(ps, cols, s_start, c_start, s_stop, c_stop, inc=None):
                n = len(cols)
                for i, (col, w) in enumerate(cols):
                    m = w // 2
                    a_p, b_p = pair(ga_t, col, w), pair(zb_t, col, w)
                    first, last = i == 0, i == n - 1
                    nc.tensor.matmul(
                        ps[0:m, 0:m], a_p, a_p,
                        start=s_start and first, stop=False, perf_mode=DR,
                    )
                    nc.tensor.matmul(
                        ps[0:m, 0:m], b_p, b_p,
                        start=False, stop=s_stop and last, perf_mode=DR,
                    )
                    nc.tensor.matmul(
                        ps[0:m, 128 : 128 + m], a_p, b_p,
                        start=c_start and first, stop=c_stop and last,
                        perf_mode=DR,
                    )
                if inc is not None:
                    # zero-cost engine op ordered after the last matmul: its
                    # sem fires without the matmul's trailing write-ack delay
                    nc.tensor.ldweights(
                        weights=pair(ga_t, cols[-1][0], cols[-1][1]),
                        perf_mode=DR,
                    ).then_inc(inc, 1)

            def bcols(b, j0, j1):
                return [(b * ROW + j * 256, 256) for j in range(j0, j1)]

            for b in range(5):
                tensor.wait_ge(sem_g[b], 16)
                tensor.wait_ge(sem_z[b], 16)
                pairs(ps_g1, bcols(b, 0, NPAIR), b == 0, b == 0, False, False)
            # KL cross-Gram: lnq is ready long before batch 5's data
            tensor.wait_ge(sem_lnq, 1)
            for k in range(KPAIR):
                mm = nc.tensor.matmul(
                    ps_kl[:], pair(qy_t, k * 256), pair(lq_t, k * 256),
                    start=k == 0, stop=k == KPAIR - 1, perf_mode=DR,
                )
            mm.then_inc(sem_peK, 1)
            tensor.wait_ge(sem_g[5], 16)
            tensor.wait_ge(sem_z[5], 16)
            pairs(ps_g1, bcols(5, 0, NPAIR), False, False, True, True,
                  inc=sem_pe1)
            tensor.wait_ge(sem_g[6], 16)
            tensor.wait_ge(sem_z[6], 16)
            pairs(ps_g2, bcols(6, 0, NPAIR), True, True, False, False)
            j0 = 0
            last = len(CH7) - 1
            for i, n in enumerate(CH7):
                tensor.wait_ge(sem_z[7 + i], 32)  # z chunk + gather share a sem
                pairs(ps_g2, bcols(7, j0, j0 + n), False, False,
                      i == last, i == last, inc=sem_pe2 if i == last else None)
                j0 += n

        @block.scalar
        def _(scalar):
            scalar.wait_ge(sem_qy, 16)
            scalar.wait_ge(sem_cp, 16)
            nc.scalar.activation(
                lq_t[:], qy_t[:], Act.Ln, bias=cp_t[:, 8:9], scale=1.0
            ).then_inc(sem_lnq, 1)
            # best term: acc[:,2] = per-partition sum(bd^2), exact f32
            scalar.wait_ge(sem_bsub, 1)
            nc.scalar.activation(
                bd_t[:], bd_t[:], Act.Square, bias=cp_t[:, 9:10],
                accum_out=acc_t[:, 2:3]
            ).then_inc(sem_bsq, 1)


        @block.vector
        def _(vector):
            state = {"nmul": 0}

            def extract(ps, width, slot, accum, sem):
                # diag(self) - 2*diag(cross) via the packed [I | -2I] mask
                scr = scr_t[:, slot * 256 : slot * 256 + width]
                nc.vector.tensor_mul(
                    scr, ps[:], mask[:, 0:width]
                ).then_inc(sem_mul, 1)
                state["nmul"] += 1
                vector.wait_ge(sem_mul, state["nmul"])  # same-engine RAW
                nc.vector.tensor_scalar(
                    out=scr,
                    in0=scr,
                    scalar1=1.0,
                    scalar2=0.0,
                    op0=Alu.mult,
                    op1=Alu.add,
                    accum_out=accum,
                ).then_inc(sem, 1)

            vector.wait_ge(sem_pe1, 1)
            extract(ps_g1, 256, 0, acc_t[:, 0:1], sem_x1)
            vector.wait_ge(sem_peK, 1)
            extract(ps_kl, 128, 1, acc_t[:, 1:2], sem_x1)
            vector.wait_ge(sem_pe2, 1)
            extract(ps_g2, 256, 2, acc_t[:, 3:4], sem_x2)

    return nc


def _get_nc(vector_dims: int):
    key = ("nc", vector_dims)
    if key not in _CACHE:
        _CACHE[key] = _build_bass(vector_dims)
    return _CACHE[key]


def _prepare(inputs):
    import ml_dtypes

    f8 = ml_dtypes.float8_e4m3
    bf16 = ml_dtypes.bfloat16

    zs = np.asarray(inputs["zs"], dtype=np.float32)
    rzs = np.asarray(inputs["rzs"], dtype=np.float32)
    pts = np.asarray(inputs["pts"], dtype=np.float32)
    pts_gt = np.asarray(inputs["pts_gt"], dtype=np.float32)
    qy = np.asarray(inputs["qy"], dtype=np.float32)
    best = np.asarray(inputs["best"], dtype=np.float64)
    best_gt = np.asarray(inputs["best_gt"], dtype=np.float64)
    mapping = np.asarray(inputs["mapping"])
    vector_dims = int(np.asarray(inputs["vector_dims"]))

    w_p = np.ones(P, dtype=np.float64)
    w_p[list(MARKS)] += W_MARK
    w_sq = np.sqrt(w_p)  # (118,) for the best term (exact f32 path)
    s_pt = (LAM * np.sqrt(w_p * D / PC)).astype(np.float32)  # fused-bank scale

    # region rows: [lam*rz | s_pt*pts | 0pad]  /  [lam*zs | s_pt*pts_gt | 0pad]
    gath8 = np.zeros((B, S, ROW), dtype=f8)
    gath8[:, :, :D] = (rzs * LAM).astype(f8)
    gath8[:, :, D : D + PC] = (pts * s_pt[None, None, :, None]).reshape(
        B, S, PC
    ).astype(f8)
    zsg8 = np.zeros((B, S, ROW), dtype=f8)
    zsg8[:, :, :D] = (zs * LAM).astype(f8)
    zsg8[:, :, D : D + PC] = (pts_gt * s_pt[None, None, :, None]).reshape(
        B, S, PC
    ).astype(f8)
    qy8 = np.ascontiguousarray((qy * np.float32(vector_dims)).astype(f8))

    best_w = (best * w_sq[None, :, None]).astype(np.float32)
    bestgt_w = (best_gt * w_sq[None, :, None]).astype(np.float32)
    mask = np.zeros((128, 256), dtype=bf16)
    mask[:, 0:128] = np.eye(128, dtype=bf16)
    mask[:, 128:256] = (-2.0 * np.eye(128)).astype(bf16)
    mask_bits = mask.view(np.float32)  # (128, 128)

    base = (np.arange(7, dtype=np.int32) * S)[:, None]  # abs offsets, b 0..6

    ch7_cols = []
    c0 = 0
    for n in CH7:
        c1 = min(c0 + n * 256, ROW)
        ch7_cols.append((c0, c1))
        c0 = c1

    in_maps = []
    for c in range(N_CORES):
        sl = slice(c * BL, (c + 1) * BL)
        m = mapping[sl].astype(np.int32)  # (8, S)
        cpk = np.zeros((S, NCONST), dtype=np.float32)
        cpk[:, 0:7] = np.ascontiguousarray((m[:7] + base).T).view(np.float32)
        cpk[:, 7] = np.ascontiguousarray(m[7]).view(np.float32)
        cpk[:, 8] = np.float32(vector_dims * EPS)
        cpk[:, 41:169] = mask_bits
        cbk = np.zeros((S, 2 * BC), dtype=np.float32)
        cbk[:P, 0:BC] = best_w[sl].transpose(1, 0, 2).reshape(P, BC)
        cbk[:P, BC : 2 * BC] = bestgt_w[sl].transpose(1, 0, 2).reshape(P, BC)
        g8 = gath8[sl]  # (8, S, ROW)
        im = {
            "gath": g8[:7].reshape(7 * S, ROW),
            "zsg": np.ascontiguousarray(
                zsg8[sl].transpose(1, 0, 2).reshape(S, BL * ROW)
            ),
            "qy8": np.ascontiguousarray(
                qy8[sl].transpose(1, 0, 2).reshape(S, BL * V)
            ),
            "cpack": cpk,
            "cbest": cbk,
        }
        for i, (c0_, c1_) in enumerate(ch7_cols):
            im[f"g7{i}"] = np.ascontiguousarray(g8[7, :, c0_:c1_])
        in_maps.append(im)
    return in_maps, vector_dims


def _combine(results, vector_dims) -> np.ndarray:
    s_fused = np.float64(0.0)
    s_kl = np.float64(0.0)
    s_best = np.float64(0.0)
    for r in results:
        por = r["po"].astype(np.float64)
        s_fused += por[:, 0].sum() + por[:, 3].sum()
        s_kl += por[:, 1].sum()
        s_best += por[:, 2].sum()

    recon_pts = s_fused / (LAM * LAM * B * S * D)
    kld = s_kl / (vector_dims * B * S)
    best_term = s_best / (B * PC)
    total = BETA * kld + GAMMA * recon_pts + best_term
    return np.float32(total)


def kernel(**inputs) -> np.ndarray:
    from concourse.bass_utils import run_bass_kernel_spmd

    in_maps, vector_dims = _prepare(inputs)
    nc = _get_nc(vector_dims)

    trace = os.environ.get("KERNEL_TRACE", "") == "1"
    res = run_bass_kernel_spmd(nc, in_maps, core_ids=list(range(N_CORES)), trace=trace)
    if trace and res.exec_time_ns is not None:
        print(f"HW exec time: {res.exec_time_ns} ns")
        if res.instructions_and_trace is not None:
            print(f"trace: {res.instructions_and_trace[1]}")

    return _combine(res.results, vector_dims)
